# revision 36
# baseline (speedup 1.0000x reference)
"""Additive-attention pooling (nn_Meta_Module) Trainium2 kernel, v2.

Full inputs in, full output out. Pure data-parallel over 8 NeuronCores
(batch 512 -> 64/core). Per core, per round of 4 batches:
  a    = all_memory @ U.T          (PE, bf16, 16 mm of [128,200] into
                                    256-col-aligned PSUM tiles)
  a+l  = DVE wide add (broadcast l per batch-segment), evict -> SBUF bf16
  t    = tanh(a+l)                 (ScalarE, 2 wide no-bias activations)
  sc   = V.T @ t                   (PE, per-batch selector column ->
                                    scores land TRANSPOSED in [32,200] PSUM)
  P    = all_memory @ MetaW.T      (PE, col-tiled partition stack as before)
  e    = exp(sc) + esum via accum_out (ScalarE, direct from PSUM)
  numer= sum_s e*P                 (PE sel-replicate + DVE mul+reduce)
Host: out = numer/esum + Metab.
"""
import os
import numpy as np
import ml_dtypes
from contextlib import ExitStack

KNOB_BCAST = int(os.environ.get("K_BCAST", "1"))
KNOB_ACCUM = int(os.environ.get("K_ACCUM", "1"))
# tensor_tensor_reduce crashes TRN2 hardware (NRT INTERNAL); default off
KNOB_TTR = int(os.environ.get("K_TTR", "0"))
KNOB_MIN = int(os.environ.get("K_MIN", "0"))  # a+bias+tanh pipeline only
KNOB_WARM = int(os.environ.get("K_WARM", "6"))  # PE warmup matmuls

import concourse.bass as bass
import concourse.tile as tile
import concourse.mybir as mybir
from concourse import bacc
from concourse.bass_utils import run_bass_kernel_spmd

BF16 = mybir.dt.bfloat16
F32 = mybir.dt.float32
AF = mybir.ActivationFunctionType
ALU = mybir.AluOpType
NBF = ml_dtypes.bfloat16

B, S, H = 512, 200, 256
N_CORES = 8
B_LOC = B // N_CORES


def build_nc(b_loc=B_LOC, debug=False):
    GROUP = 32
    N_GROUPS = b_loc // GROUP          # 2
    ROUNDS = b_loc // 4                # 16
    LAG = 2
    PREF = 4
    nc = bacc.Bacc("TRN2", target_bir_lowering=False, debug=debug)

    def din(name, shape, dt=BF16):
        return nc.dram_tensor(name, shape, dt, kind="ExternalInput")

    allT = [din(f"allT{h}", [128, b_loc * S]) for h in range(2)]
    UT_d = din("UT", [128, 512])        # U blocks (needed first)
    CB_d = din("CB", [128, 640])        # mwp 512 | sel4 128
    VS_d = din("VS", [128, 2048])       # V selector stationaries (k,j)
    LT_d = din("LT", [128, 2 * b_loc], F32)
    out_d = nc.dram_tensor("out", [128, N_GROUPS + 2], F32,
                           kind="ExternalOutput")

    with tile.TileContext(nc) as tc, ExitStack() as ctx:
        consts = ctx.enter_context(tc.tile_pool(name="consts", bufs=1))
        atp = ctx.enter_context(tc.tile_pool(name="atp", bufs=14))
        abp = ctx.enter_context(tc.tile_pool(name="abp", bufs=4))
        ttp = ctx.enter_context(tc.tile_pool(name="ttp", bufs=6))
        misc = ctx.enter_context(tc.tile_pool(name="misc", bufs=2))
        # PSUM: A tiles 3x[128,1024] (6 banks) + PT (1) + VT/erep shared (1)
        pa = ctx.enter_context(tc.tile_pool(name="pa", bufs=3, space="PSUM"))
        ppt = ctx.enter_context(tc.tile_pool(name="ppt", bufs=1, space="PSUM"))
        pvt = ctx.enter_context(tc.tile_pool(name="pvt", bufs=1, space="PSUM"))

        utt = consts.tile([128, 512], BF16, tag="c_ut")
        nc.scalar.dma_start(utt[:], UT_d.ap())
        lt = consts.tile([128, 2 * b_loc], F32, tag="c_lt")
        nc.scalar.dma_start(lt[:], LT_d.ap())
        cb = consts.tile([128, 640], BF16, tag="c_cb")
        nc.scalar.dma_start(cb[:], CB_d.ap())
        # vt_sel [128, 2*32*32]: stationary (k, j) at cols (k*32+j)*32,
        # V_k on column j, zeros elsewhere.
        vt_sel = consts.tile([128, 2048], BF16, tag="c_vt")
        nc.scalar.dma_start(vt_sel[:], VS_d.ap())
        ut = utt[:, 0:512]
        mwp = cb[:, 0:512]
        sel4 = cb[0:GROUP, 512:640]

        def UT_ap(h, o):
            return ut[:, (2 * h + o) * 128:(2 * h + o + 1) * 128]

        def MW_ap(h, i):
            return mwp[:, (h * 8 + i) * 32:(h * 8 + i) * 32 + 32]

        def VS_ap(k, j):
            return vt_sel[:, (k * 32 + j) * 32:(k * 32 + j) * 32 + 32]

        PT = ppt.tile([128, 2 * S], F32)          # P outputs, col-block per group
        # shared bank: VT (scores, transposed) at cols 0:200 (parts 0:32),
        # erep (e replicated x4) at cols 256:456
        vte = pvt.tile([128, 512], F32, name="vte")
        # combined output: cols 0:2 numer, cols 2:4 esum (rows 0:32)
        outsb = misc.tile([128, N_GROUPS + 2], F32, tag="outsb", bufs=1)
        nc.gpsimd.memset(outsb[:], 0.0)
        numer_sb = outsb[:, 0:N_GROUPS]
        esum_sb = outsb[0:GROUP, N_GROUPS:N_GROUPS + 2]
        scratch = misc.tile([128, S], BF16, tag="scr", bufs=1)
        erep_sb = misc.tile([128, S], BF16, tag="erep_sb", bufs=1)
        zjunk = misc.tile([128, 4 * S], BF16, tag="zjunk", bufs=1)
        if not KNOB_BCAST or KNOB_WARM:
            nc.gpsimd.memset(zjunk[:], 0.0)
        if KNOB_WARM:
            # Ramp the PE p-state while input DMAs are in flight: dummy
            # matmuls on a zeroed tile; results overwritten by round 0.
            warm = pa.tile([128, 1024], F32, tag="pa", name="warm")
            for _ in range(KNOB_WARM):
                nc.tensor.matmul(warm[:, 0:512], zjunk[:, 0:128],
                                 zjunk[:, 0:512], start=True, stop=True)

        at_tiles = {}
        VTs = {}
        e_sb = {}

        def dma_round(r):
            ts = []
            for h in range(2):
                a = atp.tile([128, 4 * S], BF16, tag="atp")
                nc.sync.dma_start(a[:], allT[h].ap()[:, r * 4 * S:(r + 1) * 4 * S])
                ts.append(a)
            at_tiles[r] = ts

        pt_blocks = set()

        def emit_a(r, fast_tail=False):
            # A tiles [128, 1024]: batch bb at col 256*bb (200 used)
            A = [pa.tile([128, 1024], F32, tag="pa", name=f"A{r}_{o_}")
                 for o_ in range(2)]
            at = at_tiles[r]
            for o in range(2):
                for bank in range(2):        # bank-pair: 2 batches each
                    for bb in (2 * bank, 2 * bank + 1):
                        for h in range(2):
                            first = (bb == 2 * bank) and h == 0
                            last = (bb == 2 * bank + 1) and h == 1
                            nc.tensor.matmul(
                                A[o][:, 256 * bb:256 * bb + S],
                                UT_ap(h, o),
                                at[h][:, bb * S:(bb + 1) * S],
                                start=first, stop=last)
            if fast_tail:
                # last round: per-batch biased tanh straight from PSUM —
                # higher ScalarE cost but each batch's tanh starts as soon
                # as its two matmuls land, shortening the serial tail.
                tts = []
                for o in range(2):
                    tt = ttp.tile([128, 4 * S], BF16, tag="ttp")
                    tts.append(tt)
                for bb in range(4):
                    for o in range(2):
                        nc.scalar.activation(
                            tts[o][:, bb * S:(bb + 1) * S],
                            A[o][:, 256 * bb:256 * bb + S], AF.Tanh,
                            bias=lt[:, o * b_loc + 4 * r + bb:
                                    o * b_loc + 4 * r + bb + 1])
                return tts
            # DVE: ab = A + l (broadcast per batch segment), evict to SBUF bf16
            abt = []
            for o in range(2):
                ab = abp.tile([128, 4 * S], BF16, tag="abp")
                in0 = A[o][:].rearrange("p (b c) -> p b c", b=4)[:, :, 0:S]
                if KNOB_BCAST:
                    in1 = (lt[:, o * b_loc + 4 * r:o * b_loc + 4 * r + 4]
                           .unsqueeze(2).broadcast_to((128, 4, S)))
                else:  # crash-bisect fallback: junk bias, wrong results OK
                    in1 = zjunk[:].rearrange("p (b c) -> p b c", b=4)
                out3 = ab[:].rearrange("p (b c) -> p b c", b=4)
                nc.vector.tensor_tensor(out3, in0, in1, op=ALU.add)
                abt.append(ab)
            # ScalarE: wide tanh, no bias
            tts = []
            for o in range(2):
                tt = ttp.tile([128, 4 * S], BF16, tag="ttp")
                nc.scalar.activation(tt[:], abt[o][:], AF.Tanh)
                tts.append(tt)
            return tts

        def emit_vp(r, tts):
            g = (4 * r) // GROUP
            VT = vte[0:GROUP, 0:S]
            at = at_tiles[r]
            for bb in range(4):
                bl = 4 * r + bb
                j = bl % GROUP
                for k in range(2):
                    nc.tensor.matmul(
                        VT, VS_ap(k, j), tts[k][:, bb * S:(bb + 1) * S],
                        start=(j == 0 and k == 0), stop=(j == GROUP - 1 and k == 1),
                        skip_group_check=True)
                blg = bl % GROUP
                j32 = 32 * (blg // 8)
                i8 = blg % 8
                blk_new = (g, j32) not in pt_blocks
                pt_blocks.add((g, j32))
                for h in range(2):
                    nc.tensor.matmul(
                        PT[j32:j32 + 32, g * S:(g + 1) * S],
                        MW_ap(h, i8),
                        at[h][:, bb * S:(bb + 1) * S],
                        tile_position=(0, j32),
                        start=(blk_new and h == 0), stop=(h == 1),
                        skip_group_check=True)

        def emit_exp(g):
            e = misc.tile([GROUP, S], BF16, tag=f"e{g}", bufs=1, name=f"e{g}")
            if KNOB_ACCUM:
                nc.scalar.activation(e[:], vte[0:GROUP, 0:S], AF.Exp,
                                     accum_out=esum_sb[:, g:g + 1])
            else:
                nc.scalar.activation(e[:], vte[0:GROUP, 0:S], AF.Exp)
                nc.vector.tensor_reduce(esum_sb[:, g:g + 1], e[:],
                                        axis=mybir.AxisListType.X, op=ALU.add)
            e_sb[g] = e

        def emit_numer(g):
            erep = vte[:, 256:256 + S]
            nc.tensor.matmul(erep, sel4, e_sb[g][:], start=True, stop=True)
            nc.vector.tensor_copy(erep_sb[:], erep)
            if KNOB_TTR:
                nc.vector.tensor_tensor_reduce(
                    out=scratch[:], in0=PT[:, g * S:(g + 1) * S],
                    in1=erep_sb[:], scale=1.0, scalar=0.0,
                    op0=ALU.mult, op1=ALU.add,
                    accum_out=numer_sb[:, g:g + 1])
            else:
                nc.vector.tensor_mul(scratch[:], PT[:, g * S:(g + 1) * S],
                                     erep_sb[:])
                nc.vector.tensor_reduce(numer_sb[:, g:g + 1], scratch[:],
                                        axis=mybir.AxisListType.X, op=ALU.add)

        def do_vp(rr):
            if KNOB_MIN:
                # keep outputs defined; consume tts via dummy reduce
                tts = pending.pop(rr)
                nc.vector.tensor_reduce(numer_sb[:, 0:1], tts[0][:],
                                        axis=mybir.AxisListType.X, op=ALU.add)
                del at_tiles[rr]
                return
            # numer(g-1) must precede the first V-mm of group g (erep/VT
            # share a PSUM bank; see notes above)
            if (4 * rr) % GROUP == 0 and rr > 0:
                emit_numer((4 * rr) // GROUP - 1)
            emit_vp(rr, pending.pop(rr))
            del at_tiles[rr]
            if (4 * (rr + 1)) % GROUP == 0:
                emit_exp((4 * rr) // GROUP)

        pending = {}
        for r in range(min(PREF, ROUNDS)):
            dma_round(r)
        for r in range(ROUNDS):
            if r + PREF < ROUNDS:
                dma_round(r + PREF)
            pending[r] = emit_a(r, fast_tail=(r == ROUNDS - 1))
            if r >= LAG:
                do_vp(r - LAG)
        for rr in range(ROUNDS - LAG, ROUNDS):
            do_vp(rr)
        if not KNOB_MIN:
            emit_numer(N_GROUPS - 1)
        else:
            nc.gpsimd.memset(esum_sb, 1.0)
            nc.gpsimd.memset(numer_sb[:, 1:2], 1.0)
        nc.sync.dma_start(out_d.ap(), outsb[:])
    nc.compile()
    return nc


def prep_core_inputs(all_c, last_c, U, W, V, MetaW, b_loc=B_LOC):
    GROUP = 32
    x = np.ascontiguousarray(all_c.transpose(2, 0, 1)).astype(NBF)  # [H, b, S]
    m = {}
    m["allT0"] = np.ascontiguousarray(x[:128].reshape(128, b_loc * S))
    m["allT1"] = np.ascontiguousarray(x[128:].reshape(128, b_loc * S))
    l = (last_c @ W.T).astype(np.float32)
    m["LT"] = np.ascontiguousarray(
        l.T.reshape(2, 128, b_loc).transpose(1, 0, 2).reshape(128, 2 * b_loc))
    ut = U.reshape(2, 128, 2, 128).transpose(3, 2, 0, 1).reshape(128, 512)
    mwp = np.zeros((128, 2, 8, 32), np.float32)
    for h in range(2):
        for i in range(8):
            mwp[:, h, i, 4 * i:4 * i + 4] = MetaW[:, 128 * h:128 * (h + 1)].T
    mwp = mwp.reshape(128, 512)
    sel4 = np.zeros((128, 128), np.float32)
    for mm in range(4 * GROUP):
        sel4[mm // 4, mm] = 1.0
    m["UT"] = np.ascontiguousarray(ut).astype(NBF)
    m["CB"] = np.ascontiguousarray(
        np.concatenate([mwp, sel4], axis=1)).astype(NBF)
    vs = np.zeros((128, 2, 32, 32), np.float32)
    for k in range(2):
        for j in range(32):
            vs[:, k, j, j] = V[k * 128:(k + 1) * 128, 0]
    m["VS"] = np.ascontiguousarray(vs.reshape(128, 2048)).astype(NBF)
    return m


def postprocess_core(numer, esum, Metab, b_loc=B_LOC):
    GROUP = 32
    n_groups = b_loc // GROUP
    out = np.empty((b_loc, 4), np.float32)
    for g in range(n_groups):
        out[g * GROUP:(g + 1) * GROUP] = (
            numer[:4 * GROUP, g].reshape(GROUP, 4)
            / esum[:, g].reshape(GROUP, 1))
    return out + Metab.reshape(1, 4)


_cache = {}


def _get_nc():
    if "nc" not in _cache:
        _cache["nc"] = build_nc(B_LOC)
    return _cache["nc"]


def kernel(all_memory, last_memory, U, W, V, MetaW, Metab):
    all_memory = np.asarray(all_memory, dtype=np.float32)
    last_memory = np.asarray(last_memory, dtype=np.float32)
    U = np.asarray(U, dtype=np.float32)
    W = np.asarray(W, dtype=np.float32)
    V = np.asarray(V, dtype=np.float32)
    MetaW = np.asarray(MetaW, dtype=np.float32)
    Metab = np.asarray(Metab, dtype=np.float32)
    nc = _get_nc()
    in_maps = []
    for c in range(N_CORES):
        sl = slice(c * B_LOC, (c + 1) * B_LOC)
        in_maps.append(prep_core_inputs(
            all_memory[sl], last_memory[sl], U, W, V, MetaW))
    res = run_bass_kernel_spmd(nc, in_maps, core_ids=list(range(N_CORES)))
    outs = []
    for c in range(N_CORES):
        o = res.results[c]["out"]
        outs.append(postprocess_core(o[:, 0:2], o[0:32, 2:4], Metab))
    return np.concatenate(outs, axis=0).astype(np.float32)


# revision 37
# speedup vs baseline: 1.0142x; 1.0142x over previous
"""Additive-attention pooling (nn_Meta_Module) Trainium2 kernel, v2.

Full inputs in, full output out. Pure data-parallel over 8 NeuronCores
(batch 512 -> 64/core). Per core, per round of 4 batches:
  a    = all_memory @ U.T          (PE, bf16, 16 mm of [128,200] into
                                    256-col-aligned PSUM tiles)
  a+l  = DVE wide add (broadcast l per batch-segment), evict -> SBUF bf16
  t    = tanh(a+l)                 (ScalarE, 2 wide no-bias activations)
  sc   = V.T @ t                   (PE, per-batch selector column ->
                                    scores land TRANSPOSED in [32,200] PSUM)
  P    = all_memory @ MetaW.T      (PE, col-tiled partition stack as before)
  e    = exp(sc) + esum via accum_out (ScalarE, direct from PSUM)
  numer= sum_s e*P                 (PE sel-replicate + DVE mul+reduce)
Host: out = numer/esum + Metab.
"""
import os
import numpy as np
import ml_dtypes
from contextlib import ExitStack

KNOB_BCAST = int(os.environ.get("K_BCAST", "1"))
KNOB_ACCUM = int(os.environ.get("K_ACCUM", "1"))
# tensor_tensor_reduce crashes TRN2 hardware (NRT INTERNAL); default off
KNOB_TTR = int(os.environ.get("K_TTR", "0"))
KNOB_MIN = int(os.environ.get("K_MIN", "0"))  # a+bias+tanh pipeline only
KNOB_WARM = int(os.environ.get("K_WARM", "6"))  # PE warmup matmuls

import concourse.bass as bass
import concourse.tile as tile
import concourse.mybir as mybir
from concourse import bacc
from concourse.bass_utils import run_bass_kernel_spmd

BF16 = mybir.dt.bfloat16
F32 = mybir.dt.float32
AF = mybir.ActivationFunctionType
ALU = mybir.AluOpType
NBF = ml_dtypes.bfloat16

B, S, H = 512, 200, 256
N_CORES = 8
B_LOC = B // N_CORES


def build_nc(b_loc=B_LOC, debug=False):
    GROUP = 32
    N_GROUPS = b_loc // GROUP          # 2
    ROUNDS = b_loc // 4                # 16
    LAG = 2
    PREF = 4
    nc = bacc.Bacc("TRN2", target_bir_lowering=False, debug=debug)

    def din(name, shape, dt=BF16):
        return nc.dram_tensor(name, shape, dt, kind="ExternalInput")

    allT = [din(f"allT{h}", [128, b_loc * S]) for h in range(2)]
    UT_d = din("UT", [128, 512])        # U blocks (needed first)
    CB_d = din("CB", [128, 640])        # mwp 512 | sel4 128
    VS_d = din("VS", [128, 2048])       # V selector stationaries (k,j)
    LT_d = din("LT", [128, 2 * b_loc], F32)
    out_d = nc.dram_tensor("out", [128, N_GROUPS + 2], F32,
                           kind="ExternalOutput")

    with tile.TileContext(nc) as tc, ExitStack() as ctx:
        consts = ctx.enter_context(tc.tile_pool(name="consts", bufs=1))
        atp = ctx.enter_context(tc.tile_pool(name="atp", bufs=14))
        abp = ctx.enter_context(tc.tile_pool(name="abp", bufs=4))
        ttp = ctx.enter_context(tc.tile_pool(name="ttp", bufs=6))
        misc = ctx.enter_context(tc.tile_pool(name="misc", bufs=2))
        # PSUM: A tiles 3x[128,1024] (6 banks) + PT (1) + VT/erep shared (1)
        pa = ctx.enter_context(tc.tile_pool(name="pa", bufs=3, space="PSUM"))
        ppt = ctx.enter_context(tc.tile_pool(name="ppt", bufs=1, space="PSUM"))
        pvt = ctx.enter_context(tc.tile_pool(name="pvt", bufs=1, space="PSUM"))

        utt = consts.tile([128, 512], BF16, tag="c_ut")
        nc.scalar.dma_start(utt[:], UT_d.ap())
        lt = consts.tile([128, 2 * b_loc], F32, tag="c_lt")
        nc.scalar.dma_start(lt[:], LT_d.ap())
        cb = consts.tile([128, 640], BF16, tag="c_cb")
        nc.scalar.dma_start(cb[:], CB_d.ap())
        # vt_sel [128, 2*32*32]: stationary (k, j) at cols (k*32+j)*32,
        # V_k on column j, zeros elsewhere.
        vt_sel = consts.tile([128, 2048], BF16, tag="c_vt")
        nc.scalar.dma_start(vt_sel[:], VS_d.ap())
        ut = utt[:, 0:512]
        mwp = cb[:, 0:512]
        sel4 = cb[0:GROUP, 512:640]

        def UT_ap(h, o):
            return ut[:, (2 * h + o) * 128:(2 * h + o + 1) * 128]

        def MW_ap(h, i):
            return mwp[:, (h * 8 + i) * 32:(h * 8 + i) * 32 + 32]

        def VS_ap(k, j):
            return vt_sel[:, (k * 32 + j) * 32:(k * 32 + j) * 32 + 32]

        PT = ppt.tile([128, 2 * S], F32)          # P outputs, col-block per group
        # shared bank: VT (scores, transposed) at cols 0:200 (parts 0:32),
        # erep (e replicated x4) at cols 256:456
        vte = pvt.tile([128, 512], F32, name="vte")
        # combined output: cols 0:2 numer, cols 2:4 esum (rows 0:32)
        outsb = misc.tile([128, N_GROUPS + 2], F32, tag="outsb", bufs=1)
        nc.gpsimd.memset(outsb[:], 0.0)
        numer_sb = outsb[:, 0:N_GROUPS]
        esum_sb = outsb[0:GROUP, N_GROUPS:N_GROUPS + 2]
        scratch = misc.tile([128, S], BF16, tag="scr", bufs=1)
        erep_sb = misc.tile([128, S], BF16, tag="erep_sb", bufs=1)
        zjunk = misc.tile([128, 4 * S], BF16, tag="zjunk", bufs=1)
        if not KNOB_BCAST or KNOB_WARM:
            nc.gpsimd.memset(zjunk[:], 0.0)
        if KNOB_WARM:
            # Ramp the PE p-state while input DMAs are in flight: dummy
            # matmuls on a zeroed tile; results overwritten by round 0.
            warm = pa.tile([128, 1024], F32, tag="pa", name="warm")
            for _ in range(KNOB_WARM):
                nc.tensor.matmul(warm[:, 0:512], zjunk[:, 0:128],
                                 zjunk[:, 0:512], start=True, stop=True)

        at_tiles = {}
        VTs = {}
        e_sb = {}

        def dma_round(r):
            ts = []
            for h in range(2):
                a = atp.tile([128, 4 * S], BF16, tag="atp")
                nc.sync.dma_start(a[:], allT[h].ap()[:, r * 4 * S:(r + 1) * 4 * S])
                ts.append(a)
            at_tiles[r] = ts

        pt_blocks = set()

        def emit_a(r, fast_tail=False):
            # A tiles [128, 1024]: batch bb at col 256*bb (200 used)
            A = [pa.tile([128, 1024], F32, tag="pa", name=f"A{r}_{o_}")
                 for o_ in range(2)]
            at = at_tiles[r]
            for o in range(2):
                for bank in range(2):        # bank-pair: 2 batches each
                    for bb in (2 * bank, 2 * bank + 1):
                        for h in range(2):
                            first = (bb == 2 * bank) and h == 0
                            last = (bb == 2 * bank + 1) and h == 1
                            nc.tensor.matmul(
                                A[o][:, 256 * bb:256 * bb + S],
                                UT_ap(h, o),
                                at[h][:, bb * S:(bb + 1) * S],
                                start=first, stop=last)
            if fast_tail:
                # last round: per-batch biased tanh straight from PSUM —
                # higher ScalarE cost but each batch's tanh starts as soon
                # as its two matmuls land, shortening the serial tail.
                tts = []
                for o in range(2):
                    tt = ttp.tile([128, 4 * S], BF16, tag="ttp")
                    tts.append(tt)
                for bb in range(4):
                    for o in range(2):
                        nc.scalar.activation(
                            tts[o][:, bb * S:(bb + 1) * S],
                            A[o][:, 256 * bb:256 * bb + S], AF.Tanh,
                            bias=lt[:, o * b_loc + 4 * r + bb:
                                    o * b_loc + 4 * r + bb + 1])
                return tts
            # DVE: ab = A + l (broadcast per batch segment), evict to SBUF bf16
            abt = []
            for o in range(2):
                ab = abp.tile([128, 4 * S], BF16, tag="abp")
                in0 = A[o][:].rearrange("p (b c) -> p b c", b=4)[:, :, 0:S]
                if KNOB_BCAST:
                    in1 = (lt[:, o * b_loc + 4 * r:o * b_loc + 4 * r + 4]
                           .unsqueeze(2).broadcast_to((128, 4, S)))
                else:  # crash-bisect fallback: junk bias, wrong results OK
                    in1 = zjunk[:].rearrange("p (b c) -> p b c", b=4)
                out3 = ab[:].rearrange("p (b c) -> p b c", b=4)
                nc.vector.tensor_tensor(out3, in0, in1, op=ALU.add)
                abt.append(ab)
            # ScalarE: wide tanh, no bias
            tts = []
            for o in range(2):
                tt = ttp.tile([128, 4 * S], BF16, tag="ttp")
                nc.scalar.activation(tt[:], abt[o][:], AF.Tanh)
                tts.append(tt)
            return tts

        def emit_vp(r, tts):
            g = (4 * r) // GROUP
            VT = vte[0:GROUP, 0:S]
            at = at_tiles[r]
            for bb in range(4):
                bl = 4 * r + bb
                j = bl % GROUP
                for k in range(2):
                    nc.tensor.matmul(
                        VT, VS_ap(k, j), tts[k][:, bb * S:(bb + 1) * S],
                        start=(j == 0 and k == 0), stop=(j == GROUP - 1 and k == 1),
                        skip_group_check=True)
                blg = bl % GROUP
                j32 = 32 * (blg // 8)
                i8 = blg % 8
                blk_new = (g, j32) not in pt_blocks
                pt_blocks.add((g, j32))
                for h in range(2):
                    nc.tensor.matmul(
                        PT[j32:j32 + 32, g * S:(g + 1) * S],
                        MW_ap(h, i8),
                        at[h][:, bb * S:(bb + 1) * S],
                        tile_position=(0, j32),
                        start=(blk_new and h == 0), stop=(h == 1),
                        skip_group_check=True)

        def emit_exp(g):
            e = misc.tile([GROUP, S], BF16, tag=f"e{g}", bufs=1, name=f"e{g}")
            if KNOB_ACCUM:
                nc.scalar.activation(e[:], vte[0:GROUP, 0:S], AF.Exp,
                                     accum_out=esum_sb[:, g:g + 1])
            else:
                nc.scalar.activation(e[:], vte[0:GROUP, 0:S], AF.Exp)
                nc.vector.tensor_reduce(esum_sb[:, g:g + 1], e[:],
                                        axis=mybir.AxisListType.X, op=ALU.add)
            e_sb[g] = e

        def emit_numer(g):
            erep = vte[:, 256:256 + S]
            nc.tensor.matmul(erep, sel4, e_sb[g][:], start=True, stop=True)
            nc.vector.tensor_copy(erep_sb[:], erep)
            if KNOB_TTR:
                nc.vector.tensor_tensor_reduce(
                    out=scratch[:], in0=PT[:, g * S:(g + 1) * S],
                    in1=erep_sb[:], scale=1.0, scalar=0.0,
                    op0=ALU.mult, op1=ALU.add,
                    accum_out=numer_sb[:, g:g + 1])
            else:
                nc.vector.tensor_mul(scratch[:], PT[:, g * S:(g + 1) * S],
                                     erep_sb[:])
                nc.vector.tensor_reduce(numer_sb[:, g:g + 1], scratch[:],
                                        axis=mybir.AxisListType.X, op=ALU.add)

        def do_vp(rr):
            if KNOB_MIN:
                # keep outputs defined; consume tts via dummy reduce
                tts = pending.pop(rr)
                nc.vector.tensor_reduce(numer_sb[:, 0:1], tts[0][:],
                                        axis=mybir.AxisListType.X, op=ALU.add)
                del at_tiles[rr]
                return
            # numer(g-1) must precede the first V-mm of group g (erep/VT
            # share a PSUM bank; see notes above)
            if (4 * rr) % GROUP == 0 and rr > 0:
                emit_numer((4 * rr) // GROUP - 1)
            emit_vp(rr, pending.pop(rr))
            del at_tiles[rr]
            if (4 * (rr + 1)) % GROUP == 0:
                emit_exp((4 * rr) // GROUP)

        pending = {}
        for r in range(min(PREF, ROUNDS)):
            dma_round(r)
        for r in range(ROUNDS):
            if r + PREF < ROUNDS:
                dma_round(r + PREF)
            pending[r] = emit_a(r, fast_tail=(r == 0 or r == ROUNDS - 1))
            if r >= LAG:
                do_vp(r - LAG)
        for rr in range(ROUNDS - LAG, ROUNDS):
            do_vp(rr)
        if not KNOB_MIN:
            emit_numer(N_GROUPS - 1)
        else:
            nc.gpsimd.memset(esum_sb, 1.0)
            nc.gpsimd.memset(numer_sb[:, 1:2], 1.0)
        nc.sync.dma_start(out_d.ap(), outsb[:])
    nc.compile()
    return nc


def prep_core_inputs(all_c, last_c, U, W, V, MetaW, b_loc=B_LOC):
    GROUP = 32
    x = np.ascontiguousarray(all_c.transpose(2, 0, 1)).astype(NBF)  # [H, b, S]
    m = {}
    m["allT0"] = np.ascontiguousarray(x[:128].reshape(128, b_loc * S))
    m["allT1"] = np.ascontiguousarray(x[128:].reshape(128, b_loc * S))
    l = (last_c @ W.T).astype(np.float32)
    m["LT"] = np.ascontiguousarray(
        l.T.reshape(2, 128, b_loc).transpose(1, 0, 2).reshape(128, 2 * b_loc))
    ut = U.reshape(2, 128, 2, 128).transpose(3, 2, 0, 1).reshape(128, 512)
    mwp = np.zeros((128, 2, 8, 32), np.float32)
    for h in range(2):
        for i in range(8):
            mwp[:, h, i, 4 * i:4 * i + 4] = MetaW[:, 128 * h:128 * (h + 1)].T
    mwp = mwp.reshape(128, 512)
    sel4 = np.zeros((128, 128), np.float32)
    for mm in range(4 * GROUP):
        sel4[mm // 4, mm] = 1.0
    m["UT"] = np.ascontiguousarray(ut).astype(NBF)
    m["CB"] = np.ascontiguousarray(
        np.concatenate([mwp, sel4], axis=1)).astype(NBF)
    vs = np.zeros((128, 2, 32, 32), np.float32)
    for k in range(2):
        for j in range(32):
            vs[:, k, j, j] = V[k * 128:(k + 1) * 128, 0]
    m["VS"] = np.ascontiguousarray(vs.reshape(128, 2048)).astype(NBF)
    return m


def postprocess_core(numer, esum, Metab, b_loc=B_LOC):
    GROUP = 32
    n_groups = b_loc // GROUP
    out = np.empty((b_loc, 4), np.float32)
    for g in range(n_groups):
        out[g * GROUP:(g + 1) * GROUP] = (
            numer[:4 * GROUP, g].reshape(GROUP, 4)
            / esum[:, g].reshape(GROUP, 1))
    return out + Metab.reshape(1, 4)


_cache = {}


def _get_nc():
    if "nc" not in _cache:
        _cache["nc"] = build_nc(B_LOC)
    return _cache["nc"]


def kernel(all_memory, last_memory, U, W, V, MetaW, Metab):
    all_memory = np.asarray(all_memory, dtype=np.float32)
    last_memory = np.asarray(last_memory, dtype=np.float32)
    U = np.asarray(U, dtype=np.float32)
    W = np.asarray(W, dtype=np.float32)
    V = np.asarray(V, dtype=np.float32)
    MetaW = np.asarray(MetaW, dtype=np.float32)
    Metab = np.asarray(Metab, dtype=np.float32)
    nc = _get_nc()
    in_maps = []
    for c in range(N_CORES):
        sl = slice(c * B_LOC, (c + 1) * B_LOC)
        in_maps.append(prep_core_inputs(
            all_memory[sl], last_memory[sl], U, W, V, MetaW))
    res = run_bass_kernel_spmd(nc, in_maps, core_ids=list(range(N_CORES)))
    outs = []
    for c in range(N_CORES):
        o = res.results[c]["out"]
        outs.append(postprocess_core(o[:, 0:2], o[0:32, 2:4], Metab))
    return np.concatenate(outs, axis=0).astype(np.float32)
